# revision 10
# baseline (speedup 1.0000x reference)
"""Trainium2 Bass kernel for IntrinsicMotivationManager (scatter_memory).

Pipeline (8 NeuronCores, SPMD):
  - shard rows: core c takes flattened rows [c*2048, (c+1)*2048) = batches [8c, 8c+8)
  - phase 1: DMA x in [128,2048] chunks; PE-transpose into f-major layout xT;
    bn_stats over xT gives per-feature (mean, var) partials
  - AllReduce 16KB of stats; fold normalization into projection:
    proj = x @ (inv_sigma*W) compared against threshold mproj = (mean*inv_sigma)^T W
  - phase 3: PE projection (f-contraction), sign bits, hash via powers-of-2 matmul
    producing two exact f32 16-bit halves (h_lo, h_hi) per row
  - ReduceScatter redistributes hashes so core c holds envs [8c,8c+8) over all t
  - phase 4: per-env occurrence counts via masked pairwise-equality matmul
    column sums; rewards = 1/sqrt(counts)
"""

import numpy as np
from contextlib import ExitStack

N_CORES = 8
BATCH, SEQ, FEAT, NBINS = 64, 256, 2048, 32
N = BATCH * SEQ          # 16384 flattened rows
NL = N // N_CORES        # 2048 rows per core
NCH = NL // 128          # 16 row chunks per core
NFT = FEAT // 128        # 16 feature tiles
NENV = BATCH             # 64 envs (env = i % 64)
EPV = NENV // N_CORES    # 8 envs per core
TSEQ = N // NENV         # 256 occurrences per env
TL = TSEQ // N_CORES     # 32 t-values per core per env
RMS_EPS = 1e-4

_CACHE = {}


def _build_nc(stub_cc=False):
    import concourse.bass as bass
    import concourse.bacc as bacc
    import concourse.tile as tile
    from concourse import mybir

    f32 = mybir.dt.float32
    AF = mybir.ActivationFunctionType
    ALU = mybir.AluOpType
    ds = bass.ds

    nc = bacc.Bacc("TRN2", target_bir_lowering=False, debug=False,
                   num_devices=N_CORES)

    xc = nc.dram_tensor("xc", [NL, FEAT], f32, kind="ExternalInput").ap()
    wr = nc.dram_tensor("wr", [128, NFT, NBINS], f32, kind="ExternalInput").ap()
    idn = nc.dram_tensor("idn", [128, 128], f32, kind="ExternalInput").ap()
    m01 = nc.dram_tensor("m01", [2, 128, TSEQ], f32, kind="ExternalInput").ap()
    p2d = nc.dram_tensor("p2d", [NBINS, 2], f32, kind="ExternalInput").ap()
    onesd = nc.dram_tensor("onesd", [128, 1], f32, kind="ExternalInput").ap()
    outc = nc.dram_tensor("outc", [TSEQ, EPV], f32, kind="ExternalOutput").ap()

    st_loc = nc.dram_tensor("st_loc", [128, 2 * NFT], f32).ap()
    st_sum = nc.dram_tensor("st_sum", [128, 2 * NFT], f32,
                            addr_space="Shared").ap()
    h_loc = nc.dram_tensor("h_loc", [128, TSEQ], f32).ap()
    h_rs = nc.dram_tensor("h_rs", [16, TSEQ], f32).ap()

    groups = [list(range(N_CORES))]
    n_tot = float(RMS_EPS + N)

    with tile.TileContext(nc) as tc, ExitStack() as ctx:
        const = ctx.enter_context(tc.tile_pool(name="const", bufs=1))
        chpool = ctx.enter_context(tc.tile_pool(name="ch", bufs=2))
        xtp = ctx.enter_context(tc.tile_pool(name="xt", bufs=1))
        scp = ctx.enter_context(tc.tile_pool(name="scr", bufs=2))
        smp = ctx.enter_context(tc.tile_pool(name="small", bufs=2))
        rbp = ctx.enter_context(tc.tile_pool(name="rows", bufs=2))
        ps_tp = ctx.enter_context(tc.tile_pool(name="ps_tp", bufs=2, space="PSUM"))
        ps_pr = ctx.enter_context(tc.tile_pool(name="ps_pr", bufs=2, space="PSUM"))
        ps_sm = ctx.enter_context(tc.tile_pool(name="ps_sm", bufs=2, space="PSUM"))

        sb_id = const.tile([128, 128], f32)
        nc.sync.dma_start(out=sb_id, in_=idn)
        sb_m0 = const.tile([128, TSEQ], f32)
        nc.sync.dma_start(out=sb_m0, in_=m01[0])
        sb_m1 = const.tile([128, TSEQ], f32)
        nc.sync.dma_start(out=sb_m1, in_=m01[1])
        sb_w = const.tile([128, NFT, NBINS], f32)
        nc.sync.dma_start(out=sb_w, in_=wr)
        sb_p2 = const.tile([NBINS, 2], f32)
        nc.sync.dma_start(out=sb_p2, in_=p2d)
        sb_ones = const.tile([128, 1], f32)
        nc.sync.dma_start(out=sb_ones, in_=onesd)

        xT = xtp.tile([128, NFT, NL], f32)       # xT[p, ft, n] = x[n, ft*128+p]
        bnst = const.tile([128, NFT, NCH // 4, 6], f32)
        mv = const.tile([128, NFT, 2], f32)

        # ---- phase 1: transpose + local stats ----
        for r in range(NCH):
            ch = chpool.tile([128, FEAT], f32)
            nc.sync.dma_start(out=ch, in_=xc[r * 128:(r + 1) * 128, :])
            for fg in range(NFT // 4):
                tp = ps_tp.tile([128, 512], f32)
                for q in range(4):
                    ft = 4 * fg + q
                    nc.tensor.transpose(
                        tp[:, 128 * q:128 * (q + 1)],
                        ch[:, 128 * ft:128 * (ft + 1)], sb_id)
                # one ACT copy moves 4 transposed blocks to their xT homes
                nc.scalar.copy(
                    out=xT[:, 4 * fg:4 * fg + 4, r * 128:(r + 1) * 128],
                    in_=tp.rearrange("p (q n) -> p q n", q=4))
        for ft in range(NFT):
            for nb in range(NCH // 4):
                nc.vector.bn_stats(
                    out=bnst[:, ft, nb, :],
                    in_=xT[:, ft, nb * 512:(nb + 1) * 512])
            nc.vector.bn_aggr(out=mv[:, ft, :], in_=bnst[:, ft, :, :])

        # ---- local stats -> (S1, S2) and AllReduce ----
        st_sb = const.tile([128, 2 * NFT], f32)
        lmean = mv[:, :, 0]
        lvar = mv[:, :, 1]
        nc.vector.tensor_scalar(out=st_sb[:, 0:NFT], in0=lmean,
                                scalar1=float(NL), scalar2=None, op0=ALU.mult)
        t_ms = smp.tile([128, NFT], f32)
        nc.vector.tensor_tensor(out=t_ms, in0=lmean, in1=lmean, op=ALU.mult)
        nc.vector.tensor_tensor(out=t_ms, in0=t_ms, in1=lvar, op=ALU.add)
        nc.vector.tensor_scalar(out=st_sb[:, NFT:2 * NFT], in0=t_ms,
                                scalar1=float(NL), scalar2=None, op0=ALU.mult)
        nc.sync.dma_start(out=st_loc, in_=st_sb)
        gst = const.tile([128, 2 * NFT], f32)
        if stub_cc:
            nc.sync.dma_start(out=gst, in_=st_loc)
        else:
            nc.gpsimd.collective_compute(
                "AllReduce", ALU.add, replica_groups=groups,
                ins=[st_loc], outs=[st_sum])
            nc.sync.dma_start(out=gst, in_=st_sum)

        # ---- RunningMeanStd update math (per feature) ----
        bm = const.tile([128, NFT], f32)
        nc.vector.tensor_scalar(out=bm, in0=gst[:, 0:NFT],
                                scalar1=1.0 / N, scalar2=None, op0=ALU.mult)
        tmp = smp.tile([128, NFT], f32)
        nc.vector.tensor_tensor(out=tmp, in0=gst[:, 0:NFT], in1=bm, op=ALU.mult)
        bv = const.tile([128, NFT], f32)
        nc.vector.tensor_tensor(out=bv, in0=gst[:, NFT:2 * NFT], in1=tmp,
                                op=ALU.subtract)
        nc.vector.tensor_scalar(out=bv, in0=bv, scalar1=1.0 / (N - 1),
                                scalar2=None, op0=ALU.mult)
        mean = const.tile([128, NFT], f32)
        nc.vector.tensor_scalar(out=mean, in0=bm, scalar1=float(N) / n_tot,
                                scalar2=None, op0=ALU.mult)
        # m2 = eps + bv*n + bm^2 * (eps*n/tot);  var = m2/tot; sig2 = var+1e-8
        a_t = smp.tile([128, NFT], f32)
        nc.vector.tensor_scalar(out=a_t, in0=bv, scalar1=float(N),
                                scalar2=None, op0=ALU.mult)
        b_t = smp.tile([128, NFT], f32)
        nc.vector.tensor_tensor(out=b_t, in0=bm, in1=bm, op=ALU.mult)
        nc.vector.scalar_tensor_tensor(
            out=b_t, in0=b_t, scalar=float(RMS_EPS) * N / n_tot, in1=a_t,
            op0=ALU.mult, op1=ALU.add)
        nc.vector.tensor_scalar(out=b_t, in0=b_t, scalar1=float(RMS_EPS),
                                scalar2=None, op0=ALU.add)
        sig2 = const.tile([128, NFT], f32)
        nc.vector.tensor_scalar(out=sig2, in0=b_t, scalar1=1.0 / n_tot,
                                scalar2=1e-8, op0=ALU.mult, op1=ALU.add)
        isig = const.tile([128, NFT], f32)
        nc.vector.reciprocal(out=isig, in_=sig2)
        nc.scalar.sqrt(out=isig, in_=isig)      # isig = 1/sqrt(var+1e-8)

        # ---- scaled weights and projection threshold ----
        w2 = const.tile([128, NFT, NBINS], f32)
        for ft in range(NFT):
            nc.vector.tensor_scalar(
                out=w2[:, ft, :], in0=sb_w[:, ft, :],
                scalar1=isig[:, ft:ft + 1], scalar2=None, op0=ALU.mult)
        means = const.tile([128, NFT], f32)
        nc.vector.tensor_tensor(out=means, in0=mean, in1=isig, op=ALU.mult)
        mp_ps = ps_sm.tile([NBINS, 1], f32, tag="sm")
        for ft in range(NFT):
            nc.tensor.matmul(mp_ps, w2[:, ft, :], means[:, ft:ft + 1],
                             start=(ft == 0), stop=(ft == NFT - 1))
        mproj = const.tile([NBINS, 1], f32)
        nc.scalar.copy(out=mproj, in_=mp_ps)

        # ---- phase 3: projection, sign bits, 2x16-bit hash halves ----
        # columns reordered (e, tl): local row n = 64*tl + e
        h2_sb = const.tile([2, NL], f32)
        for nb in range(4):
            pr_ps = ps_pr.tile([NBINS, 512], f32)
            for ft in range(NFT):
                rhs = xT[:, ft, :].rearrange("p (tl e) -> p e tl", e=NENV)[
                    :, nb * 16:(nb + 1) * 16, :]
                nc.tensor.matmul(pr_ps, w2[:, ft, :], rhs,
                                 start=(ft == 0), stop=(ft == NFT - 1))
            bits = scp.tile([NBINS, 512], f32)
            nc.vector.tensor_scalar(out=bits, in0=pr_ps, scalar1=mproj,
                                    scalar2=None, op0=ALU.is_gt)
            h2_ps = ps_sm.tile([2, 512], f32, tag="sm")
            nc.tensor.matmul(h2_ps, sb_p2, bits, start=True, stop=True)
            nc.scalar.copy(out=h2_sb[:, nb * 512:(nb + 1) * 512], in_=h2_ps)

        # ---- redistribute hashes by env (ReduceScatter of zero-padded slabs) --
        pid = nc.partition_id()
        hzf = const.tile([128, TSEQ], f32)   # rows (j, d, el); cols t
        nc.vector.memset(hzf, 0.0)
        for j in range(2):
            nc.gpsimd.dma_start(
                out=hzf[64 * j:64 * (j + 1), ds(pid * TL, TL)],
                in_=h2_sb[j:j + 1, :])
        hl_v = h_loc.rearrange("(d j el) t -> d j el t", j=2, el=EPV)
        for j in range(2):
            nc.sync.dma_start(out=hl_v[:, j, :, :],
                              in_=hzf[64 * j:64 * (j + 1), :])
        if stub_cc:
            nc.sync.dma_start(out=h_rs, in_=h_loc[0:16, :])
        else:
            nc.gpsimd.collective_compute(
                "ReduceScatter", ALU.add, replica_groups=groups,
                ins=[h_loc], outs=[h_rs])
        hsb_lo = const.tile([EPV, TSEQ], f32)   # rows el (this core's envs)
        hsb_hi = const.tile([EPV, TSEQ], f32)
        nc.sync.dma_start(out=hsb_lo, in_=h_rs[0:EPV, :])
        nc.sync.dma_start(out=hsb_hi, in_=h_rs[EPV:2 * EPV, :])

        # ---- phase 4: per-env occurrence counting ----
        kt = const.tile([128, 2, 2, EPV], f32)   # [t'(128), b, half, el]
        for b in range(2):
            for h in range(2):
                kt_ps = ps_sm.tile([128, EPV], f32, tag="sm")
                nc.tensor.transpose(
                    kt_ps,
                    (hsb_lo if h == 0 else hsb_hi)[:, 128 * b:128 * (b + 1)],
                    sb_id[:EPV, :EPV])
                nc.scalar.copy(out=kt[:, b, h, :], in_=kt_ps)
        csb = const.tile([1, TSEQ, EPV], f32)
        import concourse.bass as bass_mod
        for el in range(EPV):
            r_lo = rbp.tile([128, TSEQ], f32, tag="rlo")
            r_hi = rbp.tile([128, TSEQ], f32, tag="rhi")
            src_lo = h_rs[el, :]
            src_hi = h_rs[EPV + el, :]
            nc.sync.dma_start(out=r_lo, in_=bass_mod.AP(
                tensor=src_lo.tensor, offset=src_lo.offset,
                ap=[[0, 128]] + list(src_lo.ap)))
            nc.sync.dma_start(out=r_hi, in_=bass_mod.AP(
                tensor=src_hi.tensor, offset=src_hi.offset,
                ap=[[0, 128]] + list(src_hi.ap)))
            cnt_ps = ps_sm.tile([1, TSEQ], f32, tag="sm")
            for b in range(2):
                e_lo = scp.tile([128, TSEQ], f32, tag="elo")
                nc.vector.scalar_tensor_tensor(
                    out=e_lo, in0=r_lo, scalar=kt[:, b, 0, el:el + 1],
                    in1=(sb_m0 if b == 0 else sb_m1),
                    op0=ALU.is_equal, op1=ALU.mult)
                e_hi = scp.tile([128, TSEQ], f32, tag="ehi")
                nc.vector.scalar_tensor_tensor(
                    out=e_hi, in0=r_hi, scalar=kt[:, b, 1, el:el + 1],
                    in1=e_lo, op0=ALU.is_equal, op1=ALU.mult)
                nc.tensor.matmul(cnt_ps, sb_ones, e_hi,
                                 start=(b == 0), stop=(b == 1))
            nc.scalar.copy(out=csb[:, :, el], in_=cnt_ps)

        # ---- rewards = 1/sqrt(counts) ----
        csf = csb.rearrange("p t el -> p (t el)")
        nc.vector.reciprocal(out=csf, in_=csf)
        nc.scalar.sqrt(out=csf, in_=csf)
        nc.sync.dma_start(out=outc, in_=csf)

    nc.compile()
    return nc


def _host_consts():
    idn = np.eye(128, dtype=np.float32)
    t = np.arange(TSEQ)[None, :]
    tp = np.arange(128)[:, None]
    m0 = (tp <= t).astype(np.float32)
    m1 = ((128 + tp) <= t).astype(np.float32)
    m01 = np.stack([m0, m1])
    p2 = np.zeros((NBINS, 2), dtype=np.float32)
    for k in range(NBINS):
        if k < 16:
            p2[k, 0] = float(2 ** k)
        else:
            p2[k, 1] = float(2 ** (k - 16))
    ones = np.ones((128, 1), dtype=np.float32)
    return idn, m01, p2, ones


def kernel(features: np.ndarray, random_projection: np.ndarray) -> np.ndarray:
    from concourse.bass_utils import run_bass_kernel_spmd

    if "nc" not in _CACHE:
        _CACHE["nc"] = _build_nc()
    nc = _CACHE["nc"]

    feats = np.ascontiguousarray(features, dtype=np.float32)
    w = np.ascontiguousarray(random_projection, dtype=np.float32)
    wr = np.ascontiguousarray(
        w.reshape(NFT, 128, NBINS).transpose(1, 0, 2))
    idn, m01, p2, ones = _host_consts()

    in_maps = []
    for c in range(N_CORES):
        xc = np.ascontiguousarray(
            feats[EPV * c:EPV * (c + 1)].reshape(NL, FEAT))
        in_maps.append({"xc": xc, "wr": wr, "idn": idn, "m01": m01,
                        "p2d": p2, "onesd": ones})
    res = run_bass_kernel_spmd(nc, in_maps, core_ids=list(range(N_CORES)))

    out2d = np.empty((TSEQ, NENV), dtype=np.float32)
    for c in range(N_CORES):
        out2d[:, EPV * c:EPV * (c + 1)] = res.results[c]["outc"]
    return out2d.reshape(N).reshape(BATCH, SEQ, 1)


if __name__ == "__main__":
    f = np.random.randn(BATCH, SEQ, FEAT).astype(np.float32)
    w = (np.random.randn(FEAT, NBINS) / np.sqrt(FEAT)).astype(np.float32)
    out = kernel(f, w)
    print(out.shape, out.dtype, out.min(), out.max())


# revision 12
# speedup vs baseline: 1.0476x; 1.0476x over previous
"""Trainium2 Bass kernel for IntrinsicMotivationManager (scatter_memory).

Pipeline (8 NeuronCores, SPMD):
  - shard rows: core c takes flattened rows [c*2048, (c+1)*2048) = batches [8c, 8c+8)
  - phase 1: DMA x in [128,2048] chunks; PE-transpose into f-major layout xT;
    bn_stats over xT gives per-feature (mean, var) partials
  - AllReduce 16KB of stats; fold normalization into projection:
    proj = x @ (inv_sigma*W) compared against threshold mproj = (mean*inv_sigma)^T W
  - phase 3: PE projection (f-contraction), sign bits, hash via powers-of-2 matmul
    producing two exact f32 16-bit halves (h_lo, h_hi) per row
  - ReduceScatter redistributes hashes so core c holds envs [8c,8c+8) over all t
  - phase 4: per-env occurrence counts via masked pairwise-equality matmul
    column sums; rewards = 1/sqrt(counts)
"""

import numpy as np
from contextlib import ExitStack

N_CORES = 8
BATCH, SEQ, FEAT, NBINS = 64, 256, 2048, 32
N = BATCH * SEQ          # 16384 flattened rows
NL = N // N_CORES        # 2048 rows per core
NCH = NL // 128          # 16 row chunks per core
NFT = FEAT // 128        # 16 feature tiles
NENV = BATCH             # 64 envs (env = i % 64)
EPV = NENV // N_CORES    # 8 envs per core
TSEQ = N // NENV         # 256 occurrences per env
TL = TSEQ // N_CORES     # 32 t-values per core per env
RMS_EPS = 1e-4

_CACHE = {}


def _build_nc(stub_cc=False):
    import concourse.bass as bass
    import concourse.bacc as bacc
    import concourse.tile as tile
    from concourse import mybir

    f32 = mybir.dt.float32
    AF = mybir.ActivationFunctionType
    ALU = mybir.AluOpType
    ds = bass.ds

    nc = bacc.Bacc("TRN2", target_bir_lowering=False, debug=False,
                   num_devices=N_CORES)

    xc = nc.dram_tensor("xc", [NL, FEAT], f32, kind="ExternalInput").ap()
    wr = nc.dram_tensor("wr", [128, NFT, NBINS], f32, kind="ExternalInput").ap()
    idn = nc.dram_tensor("idn", [128, 128], f32, kind="ExternalInput").ap()
    m01 = nc.dram_tensor("m01", [2, 128, TSEQ], f32, kind="ExternalInput").ap()
    p2d = nc.dram_tensor("p2d", [NBINS, 2], f32, kind="ExternalInput").ap()
    onesd = nc.dram_tensor("onesd", [128, 1], f32, kind="ExternalInput").ap()
    outc = nc.dram_tensor("outc", [TSEQ, EPV], f32, kind="ExternalOutput").ap()
    dbg_h2 = nc.dram_tensor("dbg_h2", [2, NL], f32, kind="ExternalOutput").ap()
    dbg_hsb = nc.dram_tensor("dbg_hsb", [16, TSEQ], f32, kind="ExternalOutput").ap()
    dbg_cnt = nc.dram_tensor("dbg_cnt", [TSEQ, EPV], f32, kind="ExternalOutput").ap()

    st_loc = nc.dram_tensor("st_loc", [128, 2 * NFT], f32).ap()
    st_sum = nc.dram_tensor("st_sum", [128, 2 * NFT], f32,
                            addr_space="Shared").ap()
    h_loc = nc.dram_tensor("h_loc", [128, TSEQ], f32).ap()
    h_rs = nc.dram_tensor("h_rs", [16, TSEQ], f32).ap()

    groups = [list(range(N_CORES))]
    n_tot = float(RMS_EPS + N)

    with tile.TileContext(nc) as tc, ExitStack() as ctx:
        const = ctx.enter_context(tc.tile_pool(name="const", bufs=1))
        chpool = ctx.enter_context(tc.tile_pool(name="ch", bufs=2))
        xtp = ctx.enter_context(tc.tile_pool(name="xt", bufs=1))
        scp = ctx.enter_context(tc.tile_pool(name="scr", bufs=2))
        smp = ctx.enter_context(tc.tile_pool(name="small", bufs=2))
        rbp = ctx.enter_context(tc.tile_pool(name="rows", bufs=2))
        ps_tp = ctx.enter_context(tc.tile_pool(name="ps_tp", bufs=2, space="PSUM"))
        ps_pr = ctx.enter_context(tc.tile_pool(name="ps_pr", bufs=2, space="PSUM"))
        ps_sm = ctx.enter_context(tc.tile_pool(name="ps_sm", bufs=2, space="PSUM"))

        sb_id = const.tile([128, 128], f32)
        nc.sync.dma_start(out=sb_id, in_=idn)
        sb_m0 = const.tile([128, TSEQ], f32)
        nc.sync.dma_start(out=sb_m0, in_=m01[0])
        sb_m1 = const.tile([128, TSEQ], f32)
        nc.sync.dma_start(out=sb_m1, in_=m01[1])
        sb_w = const.tile([128, NFT, NBINS], f32)
        nc.sync.dma_start(out=sb_w, in_=wr)
        sb_p2 = const.tile([NBINS, 2], f32)
        nc.sync.dma_start(out=sb_p2, in_=p2d)
        sb_ones = const.tile([128, 1], f32)
        nc.sync.dma_start(out=sb_ones, in_=onesd)

        xT = xtp.tile([128, NFT, NL], f32)       # xT[p, ft, n] = x[n, ft*128+p]
        bnst = const.tile([128, NFT, NCH // 4, 6], f32)
        mv = const.tile([128, NFT, 2], f32)

        # ---- phase 1: transpose + local stats ----
        for r in range(NCH):
            ch = chpool.tile([128, FEAT], f32)
            nc.sync.dma_start(out=ch, in_=xc[r * 128:(r + 1) * 128, :])
            for fg in range(NFT // 4):
                tp = ps_tp.tile([128, 512], f32)
                for q in range(4):
                    ft = 4 * fg + q
                    nc.tensor.transpose(
                        tp[:, 128 * q:128 * (q + 1)],
                        ch[:, 128 * ft:128 * (ft + 1)], sb_id)
                # one ACT copy moves 4 transposed blocks to their xT homes
                nc.scalar.copy(
                    out=xT[:, 4 * fg:4 * fg + 4, r * 128:(r + 1) * 128],
                    in_=tp.rearrange("p (q n) -> p q n", q=4))
        for ft in range(NFT):
            for nb in range(NCH // 4):
                nc.vector.bn_stats(
                    out=bnst[:, ft, nb, :],
                    in_=xT[:, ft, nb * 512:(nb + 1) * 512])
            nc.vector.bn_aggr(out=mv[:, ft, :], in_=bnst[:, ft, :, :])

        # ---- local stats -> (S1, S2) and AllReduce ----
        st_sb = const.tile([128, 2 * NFT], f32)
        lmean = mv[:, :, 0]
        lvar = mv[:, :, 1]
        nc.vector.tensor_scalar(out=st_sb[:, 0:NFT], in0=lmean,
                                scalar1=float(NL), scalar2=None, op0=ALU.mult)
        t_ms = smp.tile([128, NFT], f32)
        nc.vector.tensor_tensor(out=t_ms, in0=lmean, in1=lmean, op=ALU.mult)
        nc.vector.tensor_tensor(out=t_ms, in0=t_ms, in1=lvar, op=ALU.add)
        nc.vector.tensor_scalar(out=st_sb[:, NFT:2 * NFT], in0=t_ms,
                                scalar1=float(NL), scalar2=None, op0=ALU.mult)
        nc.sync.dma_start(out=st_loc, in_=st_sb)
        gst = const.tile([128, 2 * NFT], f32)
        if stub_cc:
            nc.sync.dma_start(out=gst, in_=st_loc)
        else:
            nc.gpsimd.collective_compute(
                "AllReduce", ALU.add, replica_groups=groups,
                ins=[st_loc], outs=[st_sum])
            nc.sync.dma_start(out=gst, in_=st_sum)

        # ---- RunningMeanStd update math (per feature) ----
        bm = const.tile([128, NFT], f32)
        nc.vector.tensor_scalar(out=bm, in0=gst[:, 0:NFT],
                                scalar1=1.0 / N, scalar2=None, op0=ALU.mult)
        tmp = smp.tile([128, NFT], f32)
        nc.vector.tensor_tensor(out=tmp, in0=gst[:, 0:NFT], in1=bm, op=ALU.mult)
        bv = const.tile([128, NFT], f32)
        nc.vector.tensor_tensor(out=bv, in0=gst[:, NFT:2 * NFT], in1=tmp,
                                op=ALU.subtract)
        nc.vector.tensor_scalar(out=bv, in0=bv, scalar1=1.0 / (N - 1),
                                scalar2=None, op0=ALU.mult)
        mean = const.tile([128, NFT], f32)
        nc.vector.tensor_scalar(out=mean, in0=bm, scalar1=float(N) / n_tot,
                                scalar2=None, op0=ALU.mult)
        # m2 = eps + bv*n + bm^2 * (eps*n/tot);  var = m2/tot; sig2 = var+1e-8
        a_t = smp.tile([128, NFT], f32)
        nc.vector.tensor_scalar(out=a_t, in0=bv, scalar1=float(N),
                                scalar2=None, op0=ALU.mult)
        b_t = smp.tile([128, NFT], f32)
        nc.vector.tensor_tensor(out=b_t, in0=bm, in1=bm, op=ALU.mult)
        nc.vector.scalar_tensor_tensor(
            out=b_t, in0=b_t, scalar=float(RMS_EPS) * N / n_tot, in1=a_t,
            op0=ALU.mult, op1=ALU.add)
        nc.vector.tensor_scalar(out=b_t, in0=b_t, scalar1=float(RMS_EPS),
                                scalar2=None, op0=ALU.add)
        sig2 = const.tile([128, NFT], f32)
        nc.vector.tensor_scalar(out=sig2, in0=b_t, scalar1=1.0 / n_tot,
                                scalar2=1e-8, op0=ALU.mult, op1=ALU.add)
        isig = const.tile([128, NFT], f32)
        nc.vector.reciprocal(out=isig, in_=sig2)
        nc.scalar.sqrt(out=isig, in_=isig)      # isig = 1/sqrt(var+1e-8)

        # ---- scaled weights and projection threshold ----
        w2 = const.tile([128, NFT, NBINS], f32)
        for ft in range(NFT):
            nc.vector.tensor_scalar(
                out=w2[:, ft, :], in0=sb_w[:, ft, :],
                scalar1=isig[:, ft:ft + 1], scalar2=None, op0=ALU.mult)
        means = const.tile([128, NFT], f32)
        nc.vector.tensor_tensor(out=means, in0=mean, in1=isig, op=ALU.mult)
        mp_ps = ps_sm.tile([NBINS, 1], f32, tag="sm")
        for ft in range(NFT):
            nc.tensor.matmul(mp_ps, w2[:, ft, :], means[:, ft:ft + 1],
                             start=(ft == 0), stop=(ft == NFT - 1))
        mproj = const.tile([NBINS, 1], f32)
        nc.scalar.copy(out=mproj, in_=mp_ps)

        # ---- phase 3: projection, sign bits, 2x16-bit hash halves ----
        # columns reordered (e, tl): local row n = 64*tl + e
        h2f = const.tile([1, 2 * NL], f32)   # [lo cols 0:NL | hi cols NL:2NL]
        for nb in range(4):
            pr_ps = ps_pr.tile([NBINS, 512], f32)
            for ft in range(NFT):
                rhs = xT[:, ft, :].rearrange("p (tl e) -> p e tl", e=NENV)[
                    :, nb * 16:(nb + 1) * 16, :]
                nc.tensor.matmul(pr_ps, w2[:, ft, :], rhs,
                                 start=(ft == 0), stop=(ft == NFT - 1))
            bits = scp.tile([NBINS, 512], f32)
            nc.vector.tensor_scalar(out=bits, in0=pr_ps, scalar1=mproj,
                                    scalar2=None, op0=ALU.is_gt)
            for j in range(2):
                h2_ps = ps_sm.tile([1, 512], f32, tag="sm")
                nc.tensor.matmul(h2_ps, sb_p2[:, j:j + 1], bits,
                                 start=True, stop=True)
                nc.scalar.copy(
                    out=h2f[:, j * NL + nb * 512:j * NL + (nb + 1) * 512],
                    in_=h2_ps)

        # ---- redistribute hashes by env (ReduceScatter of zero-padded slabs) --
        pid = nc.partition_id()
        hzf = const.tile([128, TSEQ], f32)   # rows (j, d, el); cols t
        nc.vector.memset(hzf, 0.0)
        for j in range(2):
            nc.gpsimd.dma_start(
                out=hzf[64 * j:64 * (j + 1), ds(pid * TL, TL)],
                in_=h2f[:, j * NL:(j + 1) * NL])
        hl_v = h_loc.rearrange("(d j el) t -> d j el t", j=2, el=EPV)
        for j in range(2):
            nc.sync.dma_start(out=hl_v[:, j, :, :],
                              in_=hzf[64 * j:64 * (j + 1), :])
        if stub_cc:
            nc.sync.dma_start(out=h_rs, in_=h_loc[0:16, :])
        else:
            nc.gpsimd.collective_compute(
                "ReduceScatter", ALU.add, replica_groups=groups,
                ins=[h_loc], outs=[h_rs])
        hsb_lo = const.tile([EPV, TSEQ], f32)   # rows el (this core's envs)
        hsb_hi = const.tile([EPV, TSEQ], f32)
        nc.sync.dma_start(out=hsb_lo, in_=h_rs[0:EPV, :])
        nc.sync.dma_start(out=hsb_hi, in_=h_rs[EPV:2 * EPV, :])

        # ---- phase 4: per-env occurrence counting ----
        kt = const.tile([128, 2, 2, EPV], f32)   # [t'(128), b, half, el]
        for b in range(2):
            for h in range(2):
                kt_ps = ps_sm.tile([128, EPV], f32, tag="sm")
                nc.tensor.transpose(
                    kt_ps,
                    (hsb_lo if h == 0 else hsb_hi)[:, 128 * b:128 * (b + 1)],
                    sb_id[:EPV, :EPV])
                nc.scalar.copy(out=kt[:, b, h, :], in_=kt_ps)
        csb = const.tile([1, TSEQ, EPV], f32)
        import concourse.bass as bass_mod
        for el in range(EPV):
            r_lo = rbp.tile([128, TSEQ], f32, tag="rlo")
            r_hi = rbp.tile([128, TSEQ], f32, tag="rhi")
            src_lo = h_rs[el, :]
            src_hi = h_rs[EPV + el, :]
            nc.sync.dma_start(out=r_lo, in_=bass_mod.AP(
                tensor=src_lo.tensor, offset=src_lo.offset,
                ap=[[0, 128]] + list(src_lo.ap)))
            nc.sync.dma_start(out=r_hi, in_=bass_mod.AP(
                tensor=src_hi.tensor, offset=src_hi.offset,
                ap=[[0, 128]] + list(src_hi.ap)))
            cnt_ps = ps_sm.tile([1, TSEQ], f32, tag="sm")
            for b in range(2):
                e_lo = scp.tile([128, TSEQ], f32, tag="elo")
                nc.vector.scalar_tensor_tensor(
                    out=e_lo, in0=r_lo, scalar=kt[:, b, 0, el:el + 1],
                    in1=(sb_m0 if b == 0 else sb_m1),
                    op0=ALU.is_equal, op1=ALU.mult)
                e_hi = scp.tile([128, TSEQ], f32, tag="ehi")
                nc.vector.scalar_tensor_tensor(
                    out=e_hi, in0=r_hi, scalar=kt[:, b, 1, el:el + 1],
                    in1=e_lo, op0=ALU.is_equal, op1=ALU.mult)
                nc.tensor.matmul(cnt_ps, sb_ones, e_hi,
                                 start=(b == 0), stop=(b == 1))
            nc.scalar.copy(out=csb[:, :, el], in_=cnt_ps)

        # ---- rewards = 1/sqrt(counts) ----
        nc.sync.dma_start(out=dbg_h2,
                          in_=h2f.rearrange("p (j n) -> p j n", j=2)[0])
        nc.sync.dma_start(out=dbg_hsb[0:EPV, :], in_=hsb_lo)
        nc.sync.dma_start(out=dbg_hsb[EPV:2*EPV, :], in_=hsb_hi)
        csf = csb.rearrange("p t el -> p (t el)")
        nc.sync.dma_start(out=dbg_cnt, in_=csf.rearrange("p (t el) -> p t el", el=EPV))
        nc.vector.reciprocal(out=csf, in_=csf)
        nc.scalar.sqrt(out=csf, in_=csf)
        nc.sync.dma_start(out=outc, in_=csf)

    nc.compile()
    return nc


def _host_consts():
    idn = np.eye(128, dtype=np.float32)
    t = np.arange(TSEQ)[None, :]
    tp = np.arange(128)[:, None]
    m0 = (tp <= t).astype(np.float32)
    m1 = ((128 + tp) <= t).astype(np.float32)
    m01 = np.stack([m0, m1])
    p2 = np.zeros((NBINS, 2), dtype=np.float32)
    for k in range(NBINS):
        if k < 16:
            p2[k, 0] = float(2 ** k)
        else:
            p2[k, 1] = float(2 ** (k - 16))
    ones = np.ones((128, 1), dtype=np.float32)
    return idn, m01, p2, ones


def kernel(features: np.ndarray, random_projection: np.ndarray) -> np.ndarray:
    from concourse.bass_utils import run_bass_kernel_spmd

    if "nc" not in _CACHE:
        _CACHE["nc"] = _build_nc()
    nc = _CACHE["nc"]

    feats = np.ascontiguousarray(features, dtype=np.float32)
    w = np.ascontiguousarray(random_projection, dtype=np.float32)
    wr = np.ascontiguousarray(
        w.reshape(NFT, 128, NBINS).transpose(1, 0, 2))
    idn, m01, p2, ones = _host_consts()

    in_maps = []
    for c in range(N_CORES):
        xc = np.ascontiguousarray(
            feats[EPV * c:EPV * (c + 1)].reshape(NL, FEAT))
        in_maps.append({"xc": xc, "wr": wr, "idn": idn, "m01": m01,
                        "p2d": p2, "onesd": ones})
    res = run_bass_kernel_spmd(nc, in_maps, core_ids=list(range(N_CORES)))

    out2d = np.empty((TSEQ, NENV), dtype=np.float32)
    for c in range(N_CORES):
        out2d[:, EPV * c:EPV * (c + 1)] = res.results[c]["outc"]
    return out2d.reshape(N).reshape(BATCH, SEQ, 1)


if __name__ == "__main__":
    f = np.random.randn(BATCH, SEQ, FEAT).astype(np.float32)
    w = (np.random.randn(FEAT, NBINS) / np.sqrt(FEAT)).astype(np.float32)
    out = kernel(f, w)
    print(out.shape, out.dtype, out.min(), out.max())


# revision 13
# speedup vs baseline: 23555.1154x; 22485.2748x over previous
"""Trainium2 Bass kernel for IntrinsicMotivationManager (scatter_memory).

Pipeline (8 NeuronCores, SPMD):
  - shard rows: core c takes flattened rows [c*2048, (c+1)*2048) = batches [8c, 8c+8)
  - phase 1: DMA x in [128,2048] chunks; PE-transpose into f-major layout xT;
    bn_stats over xT gives per-feature (mean, var) partials
  - AllReduce 16KB of stats; fold normalization into projection:
    proj = x @ (inv_sigma*W) compared against threshold mproj = (mean*inv_sigma)^T W
  - phase 3: PE projection (f-contraction), sign bits, hash via powers-of-2 matmul
    producing two exact f32 16-bit halves (h_lo, h_hi) per row
  - ReduceScatter redistributes hashes so core c holds envs [8c,8c+8) over all t
  - phase 4: per-env occurrence counts via masked pairwise-equality matmul
    column sums; rewards = 1/sqrt(counts)
"""

import numpy as np
from contextlib import ExitStack

N_CORES = 8
BATCH, SEQ, FEAT, NBINS = 64, 256, 2048, 32
N = BATCH * SEQ          # 16384 flattened rows
NL = N // N_CORES        # 2048 rows per core
NCH = NL // 128          # 16 row chunks per core
NFT = FEAT // 128        # 16 feature tiles
NENV = BATCH             # 64 envs (env = i % 64)
EPV = NENV // N_CORES    # 8 envs per core
TSEQ = N // NENV         # 256 occurrences per env
TL = TSEQ // N_CORES     # 32 t-values per core per env
RMS_EPS = 1e-4

_CACHE = {}


def _build_nc(stub_cc=False):
    import concourse.bass as bass
    import concourse.bacc as bacc
    import concourse.tile as tile
    from concourse import mybir

    f32 = mybir.dt.float32
    AF = mybir.ActivationFunctionType
    ALU = mybir.AluOpType
    ds = bass.ds

    nc = bacc.Bacc("TRN2", target_bir_lowering=False, debug=False,
                   num_devices=N_CORES)

    xc = nc.dram_tensor("xc", [NL, FEAT], f32, kind="ExternalInput").ap()
    wr = nc.dram_tensor("wr", [128, NFT, NBINS], f32, kind="ExternalInput").ap()
    idn = nc.dram_tensor("idn", [128, 128], f32, kind="ExternalInput").ap()
    m01 = nc.dram_tensor("m01", [2, 128, TSEQ], f32, kind="ExternalInput").ap()
    p2d = nc.dram_tensor("p2d", [NBINS, 2], f32, kind="ExternalInput").ap()
    onesd = nc.dram_tensor("onesd", [128, 1], f32, kind="ExternalInput").ap()
    outc = nc.dram_tensor("outc", [TSEQ, EPV], f32, kind="ExternalOutput").ap()
    dbg_h2 = nc.dram_tensor("dbg_h2", [2, NL], f32, kind="ExternalOutput").ap()
    dbg_hsb = nc.dram_tensor("dbg_hsb", [16, TSEQ], f32, kind="ExternalOutput").ap()
    dbg_cnt = nc.dram_tensor("dbg_cnt", [TSEQ, EPV], f32, kind="ExternalOutput").ap()

    st_loc = nc.dram_tensor("st_loc", [128, 2 * NFT], f32).ap()
    st_sum = nc.dram_tensor("st_sum", [128, 2 * NFT], f32,
                            addr_space="Shared").ap()
    h_loc = nc.dram_tensor("h_loc", [128, TSEQ], f32).ap()
    h_rs = nc.dram_tensor("h_rs", [16, TSEQ], f32).ap()

    groups = [list(range(N_CORES))]
    n_tot = float(RMS_EPS + N)

    with tile.TileContext(nc) as tc, ExitStack() as ctx:
        const = ctx.enter_context(tc.tile_pool(name="const", bufs=1))
        chpool = ctx.enter_context(tc.tile_pool(name="ch", bufs=2))
        xtp = ctx.enter_context(tc.tile_pool(name="xt", bufs=1))
        scp = ctx.enter_context(tc.tile_pool(name="scr", bufs=2))
        smp = ctx.enter_context(tc.tile_pool(name="small", bufs=2))
        rbp = ctx.enter_context(tc.tile_pool(name="rows", bufs=2))
        ps_tp = ctx.enter_context(tc.tile_pool(name="ps_tp", bufs=2, space="PSUM"))
        ps_pr = ctx.enter_context(tc.tile_pool(name="ps_pr", bufs=2, space="PSUM"))
        ps_sm = ctx.enter_context(tc.tile_pool(name="ps_sm", bufs=2, space="PSUM"))

        sb_id = const.tile([128, 128], f32)
        nc.sync.dma_start(out=sb_id, in_=idn)
        sb_m0 = const.tile([128, TSEQ], f32)
        nc.sync.dma_start(out=sb_m0, in_=m01[0])
        sb_m1 = const.tile([128, TSEQ], f32)
        nc.sync.dma_start(out=sb_m1, in_=m01[1])
        sb_w = const.tile([128, NFT, NBINS], f32)
        nc.sync.dma_start(out=sb_w, in_=wr)
        sb_p2 = const.tile([NBINS, 2], f32)
        nc.sync.dma_start(out=sb_p2, in_=p2d)
        sb_ones = const.tile([128, 1], f32)
        nc.sync.dma_start(out=sb_ones, in_=onesd)

        xT = xtp.tile([128, NFT, NL], f32)       # xT[p, ft, n] = x[n, ft*128+p]
        bnst = const.tile([128, NFT, NCH // 4, 6], f32)
        mv = const.tile([128, NFT, 2], f32)

        # ---- phase 1: transpose + local stats ----
        for r in range(NCH):
            ch = chpool.tile([128, FEAT], f32)
            nc.sync.dma_start(out=ch, in_=xc[r * 128:(r + 1) * 128, :])
            for fg in range(NFT // 4):
                tp = ps_tp.tile([128, 512], f32)
                for q in range(4):
                    ft = 4 * fg + q
                    nc.tensor.transpose(
                        tp[:, 128 * q:128 * (q + 1)],
                        ch[:, 128 * ft:128 * (ft + 1)], sb_id)
                # one ACT copy moves 4 transposed blocks to their xT homes
                nc.scalar.copy(
                    out=xT[:, 4 * fg:4 * fg + 4, r * 128:(r + 1) * 128],
                    in_=tp.rearrange("p (q n) -> p q n", q=4))
        for ft in range(NFT):
            for nb in range(NCH // 4):
                nc.vector.bn_stats(
                    out=bnst[:, ft, nb, :],
                    in_=xT[:, ft, nb * 512:(nb + 1) * 512])
            nc.vector.bn_aggr(out=mv[:, ft, :], in_=bnst[:, ft, :, :])

        # ---- local stats -> (S1, S2) and AllReduce ----
        st_sb = const.tile([128, 2 * NFT], f32)
        lmean = mv[:, :, 0]
        lvar = mv[:, :, 1]
        nc.vector.tensor_scalar(out=st_sb[:, 0:NFT], in0=lmean,
                                scalar1=float(NL), scalar2=None, op0=ALU.mult)
        t_ms = smp.tile([128, NFT], f32)
        nc.vector.tensor_tensor(out=t_ms, in0=lmean, in1=lmean, op=ALU.mult)
        nc.vector.tensor_tensor(out=t_ms, in0=t_ms, in1=lvar, op=ALU.add)
        nc.vector.tensor_scalar(out=st_sb[:, NFT:2 * NFT], in0=t_ms,
                                scalar1=float(NL), scalar2=None, op0=ALU.mult)
        nc.sync.dma_start(out=st_loc, in_=st_sb)
        gst = const.tile([128, 2 * NFT], f32)
        if stub_cc:
            nc.sync.dma_start(out=gst, in_=st_loc)
        else:
            nc.gpsimd.collective_compute(
                "AllReduce", ALU.add, replica_groups=groups,
                ins=[st_loc], outs=[st_sum])
            nc.sync.dma_start(out=gst, in_=st_sum)

        # ---- RunningMeanStd update math (per feature) ----
        bm = const.tile([128, NFT], f32)
        nc.vector.tensor_scalar(out=bm, in0=gst[:, 0:NFT],
                                scalar1=1.0 / N, scalar2=None, op0=ALU.mult)
        tmp = smp.tile([128, NFT], f32)
        nc.vector.tensor_tensor(out=tmp, in0=gst[:, 0:NFT], in1=bm, op=ALU.mult)
        bv = const.tile([128, NFT], f32)
        nc.vector.tensor_tensor(out=bv, in0=gst[:, NFT:2 * NFT], in1=tmp,
                                op=ALU.subtract)
        nc.vector.tensor_scalar(out=bv, in0=bv, scalar1=1.0 / (N - 1),
                                scalar2=None, op0=ALU.mult)
        mean = const.tile([128, NFT], f32)
        nc.vector.tensor_scalar(out=mean, in0=bm, scalar1=float(N) / n_tot,
                                scalar2=None, op0=ALU.mult)
        # m2 = eps + bv*n + bm^2 * (eps*n/tot);  var = m2/tot; sig2 = var+1e-8
        a_t = smp.tile([128, NFT], f32)
        nc.vector.tensor_scalar(out=a_t, in0=bv, scalar1=float(N),
                                scalar2=None, op0=ALU.mult)
        b_t = smp.tile([128, NFT], f32)
        nc.vector.tensor_tensor(out=b_t, in0=bm, in1=bm, op=ALU.mult)
        nc.vector.scalar_tensor_tensor(
            out=b_t, in0=b_t, scalar=float(RMS_EPS) * N / n_tot, in1=a_t,
            op0=ALU.mult, op1=ALU.add)
        nc.vector.tensor_scalar(out=b_t, in0=b_t, scalar1=float(RMS_EPS),
                                scalar2=None, op0=ALU.add)
        sig2 = const.tile([128, NFT], f32)
        nc.vector.tensor_scalar(out=sig2, in0=b_t, scalar1=1.0 / n_tot,
                                scalar2=1e-8, op0=ALU.mult, op1=ALU.add)
        isig = const.tile([128, NFT], f32)
        nc.vector.reciprocal(out=isig, in_=sig2)
        nc.scalar.sqrt(out=isig, in_=isig)      # isig = 1/sqrt(var+1e-8)

        # ---- scaled weights and projection threshold ----
        w2 = const.tile([128, NFT, NBINS], f32)
        for ft in range(NFT):
            nc.vector.tensor_scalar(
                out=w2[:, ft, :], in0=sb_w[:, ft, :],
                scalar1=isig[:, ft:ft + 1], scalar2=None, op0=ALU.mult)
        means = const.tile([128, NFT], f32)
        nc.vector.tensor_tensor(out=means, in0=mean, in1=isig, op=ALU.mult)
        mp_ps = ps_sm.tile([NBINS, 1], f32, tag="sm")
        for ft in range(NFT):
            nc.tensor.matmul(mp_ps, w2[:, ft, :], means[:, ft:ft + 1],
                             start=(ft == 0), stop=(ft == NFT - 1))
        mproj = const.tile([NBINS, 1], f32)
        nc.scalar.copy(out=mproj, in_=mp_ps)

        # ---- phase 3: projection, sign bits, 2x16-bit hash halves ----
        # columns reordered (e, tl): local row n = 64*tl + e
        h2f = const.tile([1, 2 * NL], f32)   # [lo cols 0:NL | hi cols NL:2NL]
        for nb in range(4):
            pr_ps = ps_pr.tile([NBINS, 512], f32)
            for ft in range(NFT):
                rhs = xT[:, ft, :].rearrange("p (tl e) -> p e tl", e=NENV)[
                    :, nb * 16:(nb + 1) * 16, :]
                nc.tensor.matmul(pr_ps, w2[:, ft, :], rhs,
                                 start=(ft == 0), stop=(ft == NFT - 1))
            bits = scp.tile([NBINS, 512], f32)
            nc.vector.tensor_scalar(out=bits, in0=pr_ps, scalar1=mproj,
                                    scalar2=None, op0=ALU.is_gt)
            for j in range(2):
                h2_ps = ps_sm.tile([1, 512], f32, tag="sm")
                nc.tensor.matmul(h2_ps, sb_p2[:, j:j + 1], bits,
                                 start=True, stop=True)
                nc.scalar.copy(
                    out=h2f[:, j * NL + nb * 512:j * NL + (nb + 1) * 512],
                    in_=h2_ps)

        # ---- redistribute hashes by env (ReduceScatter of zero-padded slabs) --
        pid = nc.partition_id()
        hzf = const.tile([128, TSEQ], f32)   # rows (j, d, el); cols t
        nc.vector.memset(hzf, 0.0)
        nc.gpsimd.dma_start(out=hzf[:, ds(pid * TL, TL)], in_=h2f)
        hl_v = h_loc.rearrange("(d j el) t -> d j el t", j=2, el=EPV)
        for j in range(2):
            nc.sync.dma_start(out=hl_v[:, j, :, :],
                              in_=hzf[64 * j:64 * (j + 1), :])
        if stub_cc:
            nc.sync.dma_start(out=h_rs, in_=h_loc[0:16, :])
        else:
            nc.gpsimd.collective_compute(
                "ReduceScatter", ALU.add, replica_groups=groups,
                ins=[h_loc], outs=[h_rs])
        hsb_lo = const.tile([EPV, TSEQ], f32)   # rows el (this core's envs)
        hsb_hi = const.tile([EPV, TSEQ], f32)
        nc.sync.dma_start(out=hsb_lo, in_=h_rs[0:EPV, :])
        nc.sync.dma_start(out=hsb_hi, in_=h_rs[EPV:2 * EPV, :])

        # ---- phase 4: per-env occurrence counting ----
        kt = const.tile([128, 2, 2, EPV], f32)   # [t'(128), b, half, el]
        for b in range(2):
            for h in range(2):
                kt_ps = ps_sm.tile([128, EPV], f32, tag="sm")
                nc.tensor.transpose(
                    kt_ps,
                    (hsb_lo if h == 0 else hsb_hi)[:, 128 * b:128 * (b + 1)],
                    sb_id[:EPV, :EPV])
                nc.scalar.copy(out=kt[:, b, h, :], in_=kt_ps)
        csb = const.tile([1, TSEQ, EPV], f32)
        import concourse.bass as bass_mod
        for el in range(EPV):
            r_lo = rbp.tile([128, TSEQ], f32, tag="rlo")
            r_hi = rbp.tile([128, TSEQ], f32, tag="rhi")
            src_lo = h_rs[el, :]
            src_hi = h_rs[EPV + el, :]
            nc.sync.dma_start(out=r_lo, in_=bass_mod.AP(
                tensor=src_lo.tensor, offset=src_lo.offset,
                ap=[[0, 128]] + list(src_lo.ap)))
            nc.sync.dma_start(out=r_hi, in_=bass_mod.AP(
                tensor=src_hi.tensor, offset=src_hi.offset,
                ap=[[0, 128]] + list(src_hi.ap)))
            cnt_ps = ps_sm.tile([1, TSEQ], f32, tag="sm")
            for b in range(2):
                e_lo = scp.tile([128, TSEQ], f32, tag="elo")
                nc.vector.scalar_tensor_tensor(
                    out=e_lo, in0=r_lo, scalar=kt[:, b, 0, el:el + 1],
                    in1=(sb_m0 if b == 0 else sb_m1),
                    op0=ALU.is_equal, op1=ALU.mult)
                e_hi = scp.tile([128, TSEQ], f32, tag="ehi")
                nc.vector.scalar_tensor_tensor(
                    out=e_hi, in0=r_hi, scalar=kt[:, b, 1, el:el + 1],
                    in1=e_lo, op0=ALU.is_equal, op1=ALU.mult)
                nc.tensor.matmul(cnt_ps, sb_ones, e_hi,
                                 start=(b == 0), stop=(b == 1))
            nc.scalar.copy(out=csb[:, :, el], in_=cnt_ps)

        # ---- rewards = 1/sqrt(counts) ----
        nc.sync.dma_start(out=dbg_h2,
                          in_=h2f.rearrange("p (j n) -> p j n", j=2)[0])
        nc.sync.dma_start(out=dbg_hsb[0:EPV, :], in_=hsb_lo)
        nc.sync.dma_start(out=dbg_hsb[EPV:2*EPV, :], in_=hsb_hi)
        csf = csb.rearrange("p t el -> p (t el)")
        nc.sync.dma_start(out=dbg_cnt, in_=csf.rearrange("p (t el) -> p t el", el=EPV))
        nc.vector.reciprocal(out=csf, in_=csf)
        nc.scalar.sqrt(out=csf, in_=csf)
        nc.sync.dma_start(out=outc, in_=csf)

    nc.compile()
    return nc


def _host_consts():
    idn = np.eye(128, dtype=np.float32)
    t = np.arange(TSEQ)[None, :]
    tp = np.arange(128)[:, None]
    m0 = (tp <= t).astype(np.float32)
    m1 = ((128 + tp) <= t).astype(np.float32)
    m01 = np.stack([m0, m1])
    p2 = np.zeros((NBINS, 2), dtype=np.float32)
    for k in range(NBINS):
        if k < 16:
            p2[k, 0] = float(2 ** k)
        else:
            p2[k, 1] = float(2 ** (k - 16))
    ones = np.ones((128, 1), dtype=np.float32)
    return idn, m01, p2, ones


def kernel(features: np.ndarray, random_projection: np.ndarray) -> np.ndarray:
    from concourse.bass_utils import run_bass_kernel_spmd

    if "nc" not in _CACHE:
        _CACHE["nc"] = _build_nc()
    nc = _CACHE["nc"]

    feats = np.ascontiguousarray(features, dtype=np.float32)
    w = np.ascontiguousarray(random_projection, dtype=np.float32)
    wr = np.ascontiguousarray(
        w.reshape(NFT, 128, NBINS).transpose(1, 0, 2))
    idn, m01, p2, ones = _host_consts()

    in_maps = []
    for c in range(N_CORES):
        xc = np.ascontiguousarray(
            feats[EPV * c:EPV * (c + 1)].reshape(NL, FEAT))
        in_maps.append({"xc": xc, "wr": wr, "idn": idn, "m01": m01,
                        "p2d": p2, "onesd": ones})
    res = run_bass_kernel_spmd(nc, in_maps, core_ids=list(range(N_CORES)))

    out2d = np.empty((TSEQ, NENV), dtype=np.float32)
    for c in range(N_CORES):
        out2d[:, EPV * c:EPV * (c + 1)] = res.results[c]["outc"]
    return out2d.reshape(N).reshape(BATCH, SEQ, 1)


if __name__ == "__main__":
    f = np.random.randn(BATCH, SEQ, FEAT).astype(np.float32)
    w = (np.random.randn(FEAT, NBINS) / np.sqrt(FEAT)).astype(np.float32)
    out = kernel(f, w)
    print(out.shape, out.dtype, out.min(), out.max())
